# revision 20
# baseline (speedup 1.0000x reference)
"""Trainium2 Bass kernel for nn_DIAGCN (RGCN + GraphConv + classifier over
block-diagonal dialog graphs), SPMD over 8 NeuronCores.

Strategy
--------
The dialog graph is a causal 5-tap window (edges i -> i+o, o = 0..4, within
each 100-utterance dialog), and relation_type(i,j) = spk[i]*spk[j] with spk
derived from self-edges.  Every per-node linear map commutes with both the
window sum W(.) (row-mixing) and per-node diagonal scalings (row scaling), so
the whole network folds into 7-wide channels:

    y   = W(g0) + f0 + const
    g0  = rootA + ic0.*W(q0A) + (ic0-ic0s).*W(m0A) + ic1s.*W(m1A)
    f0  = same with B-weights
    m{0,1} = spk.*u{0,1},  q0 = (1-spk).*u0,  u{0,1}{A,B} = x@(w_rel{0,1}@w{A,B})
    wA = w_gc_rel@w_clf,  wB = w_gc_root@w_clf
    roots = x @ (w_root@w{A,B} [+ w_skip@w_clf])
    const = cA*W(mask) + cBc*mask  (all bias terms, host-precomputed)

Device work per core: one [1024 -> 78] fp16 matmul over x; ONE DVE op per
tile evacuates PSUM to fp16 SBUF with per-row masks applied (root rows get
the data mask); 5-tap shift-tree windows + coef multiply on DVE (fp16 2x
mode); a [78 -> 39] selection matmul pair, lagged TWO tiles and issued at
the head of each iteration so the in-order PE queue never drains (a fed PE
streams 512-col matmuls at ~215 ns; a stalled one halves its rate), reduces
channels to packed g0/f0 half planes; a second packed window over g0 runs
in column chunks spread across the loop so the serial tail stays short.
Output flush DMAs ride the Activation issue queue so the sync queue paces
xt tiles undisturbed.  All HBM traffic is fp16 (the 2e-2 rel
tolerance leaves ample room), halving memory-bound DMA bytes vs f32, with
8 KiB contiguous per partition per xt DMA.

Layout: nodes sharded by dialog (no cross-core edges), 64 padded dialogs per
core; each dialog stored as 4 zero "gap" columns + 100 data columns so window
sums never leak across dialogs.
"""
import numpy as np

# ---------------------------------------------------------------- constants
B, L, FUT = 500, 100, 4
N = B * L
IN, HID, NCLS = 1024, 512, 7
NCORES = 8
GAP = 4
DLG = L + GAP            # 104 columns per dialog
DPC = 64                 # padded dialogs per core
COLS = DPC * DLG         # 6656 columns per core
NT = 13                  # column tiles
NTC = COLS // NT         # 512
KB = IN // 128           # 8 contraction blocks
M = 78                   # Wbig columns (psum partitions used)
M2 = 39                  # S-matmul output columns
GRP = COLS // 4          # 1664 columns per packed win2 group (16 dialogs)
CH = GRP // 4            # win2 column-chunk size (4 dialogs)
NR = 96 + NCLS           # packed plane rows
WROWS = 42               # windowed rows (real)
VROWS = 64               # rows carried through the window tiles

D_COUNTS = [63, 63, 63, 63, 62, 62, 62, 62]
D_STARTS = np.concatenate([[0], np.cumsum(D_COUNTS)])[:-1]

# psum/Z row map: 0:7 u0A, 7:14 u0B (-> m0 = spk.*u0), 14:21 u1A, 21:28 u1B
# (-> m1 = spk.*u1), 28:35 u0A dup, 35:42 u0B dup (-> q0 = (1-spk).*u0),
# rows 42:64 zero, 64:71 rootA, 71:78 rootB (mask-scaled copy).


def _data_cols():
    d = np.arange(DPC)[:, None]
    u = np.arange(L)[None, :]
    return d * DLG + GAP + u  # [DPC, L]


# ---------------------------------------------------------------- host prep
def _check_graph(edges, relation_type):
    i = np.arange(L)[:, None]
    off = np.arange(FUT + 1)[None, :]
    tl = i + off
    valid = tl < L
    sl = np.broadcast_to(i, tl.shape)[valid]
    tl = tl[valid]
    base = (np.arange(B) * L)[:, None]
    src = (base + sl[None, :]).reshape(-1)
    tgt = (base + tl[None, :]).reshape(-1)
    if edges.shape != (2, src.size) or not (
        np.array_equal(edges[0], src) and np.array_equal(edges[1], tgt)
    ):
        raise ValueError("edge structure does not match the DIAGCN pattern")
    sel = edges[0] == edges[1]
    spk = np.zeros(N, dtype=np.float64)
    spk[edges[0][sel]] = relation_type[sel]
    return spk


def _win_cols(v):
    """5-tap causal window along the padded column axis."""
    out = np.zeros_like(v)
    for o in range(FUT + 1):
        if o:
            out[o:] += v[:v.shape[0] - o]
        else:
            out += v
    return out


def _host_prep(x, edges, relation_type, w_rel, w_root, b_rgcn,
               w_gc_rel, w_gc_root, b_gc, w_skip, b_skip, w_clf, b_clf):
    x = np.asarray(x, dtype=np.float32)
    edges = np.asarray(edges)
    relation_type = np.asarray(relation_type)
    spk = _check_graph(edges, relation_type)

    tgt = edges[1]
    c1 = np.bincount(tgt[relation_type == 1], minlength=N).astype(np.float64)
    c0 = np.bincount(tgt[relation_type == 0], minlength=N).astype(np.float64)
    ic0 = 1.0 / np.maximum(c0, 1.0)
    ic1 = 1.0 / np.maximum(c1, 1.0)
    ic0s = ic0 * spk
    ic1s = ic1 * spk

    f8 = lambda a: np.asarray(a, dtype=np.float64)
    w_rel, w_root, w_gc_rel, w_gc_root, w_skip, w_clf = map(
        f8, (w_rel, w_root, w_gc_rel, w_gc_root, w_skip, w_clf))
    b_rgcn, b_gc, b_skip, b_clf = map(f8, (b_rgcn, b_gc, b_skip, b_clf))

    wA = w_gc_rel @ w_clf
    wB = w_gc_root @ w_clf
    w0A, w1A = w_rel[0] @ wA, w_rel[1] @ wA
    w0B, w1B = w_rel[0] @ wB, w_rel[1] @ wB
    Wbig = np.zeros((IN, M), dtype=np.float64)
    Wbig[:, 0:7] = w0A
    Wbig[:, 7:14] = w0B
    Wbig[:, 14:21] = w1A
    Wbig[:, 21:28] = w1B
    Wbig[:, 28:35] = w0A
    Wbig[:, 35:42] = w0B
    Wbig[:, 64:71] = w_root @ wA
    Wbig[:, 71:78] = w_root @ wB + w_skip @ w_clf
    # [128 partitions, KB, M]: partition p holds weight rows {k*128+p}
    Wbig = np.ascontiguousarray(
        Wbig.astype(np.float16).reshape(KB, 128, M).swapaxes(0, 1))

    # S matrices: channel reduction onto ps2 rows {0:7}=g0, {32:39}=f0;
    # slot 0 reduces the windowed+coef'd V rows, slot 1 picks the roots.
    S = np.zeros((128, 2, M2), dtype=np.float16)
    for i in range(7):
        for r in (0, 14, 28):               # m0, m1, q0 (windowed, coef'd)
            S[r + i, 0, i] = 1.0            # A-channels -> g0
            S[r + 7 + i, 0, 32 + i] = 1.0   # B-channels -> f0
        S[64 + i, 1, i] = 1.0               # rootA -> g0
        S[71 + i, 1, 32 + i] = 1.0          # rootB -> f0

    cA = b_rgcn @ wA                        # [7]
    cBc = b_rgcn @ wB + (b_gc + b_skip) @ w_clf + b_clf

    dc = _data_cols()
    mask_col = np.zeros(COLS, dtype=np.float64)
    mask_col[dc.reshape(-1)] = 1.0
    wmask = _win_cols(mask_col)
    # out_const = cA (x) W(mask) + cBc (x) mask; one packed plane with
    # quarter g at rows 32g:32g+7.
    constp = np.zeros((NR, GRP), dtype=np.float16)
    for g in range(4):
        seg = slice(g * GRP, (g + 1) * GRP)
        constp[32 * g:32 * g + 7] = (
            cA[:, None] * wmask[None, seg] + cBc[:, None] * mask_col[None, seg]
        ).astype(np.float16)

    in_maps = []
    unshard_info = []
    for c in range(NCORES):
        nd = D_COUNTS[c]
        g0 = D_STARTS[c]
        cols_real = dc[:nd].reshape(-1)
        nodes_real = g0 * L + np.arange(nd * L)

        xt = np.zeros((IN, COLS), dtype=np.float16)
        xt[:, cols_real] = x[nodes_real].T.astype(np.float16)
        # dram layout [NT, 128, KB*NTC]: one DMA per tile, 8 KiB contiguous
        # per partition (8 k-blocks x 512 cols fp16)
        xts = np.ascontiguousarray(
            xt.reshape(KB, 128, NT, NTC).transpose(2, 1, 0, 3)
            .reshape(NT, 128, KB * NTC))

        def vec_to_cols(v):
            out = np.zeros(COLS, dtype=np.float64)
            out[cols_real] = v[nodes_real]
            return out

        spk_c = vec_to_cols(spk)
        ic0_c = vec_to_cols(ic0)
        ic0s_c = vec_to_cols(ic0s)
        ic1s_c = vec_to_cols(ic1s)

        spkrep = np.zeros((M, COLS), dtype=np.float16)
        spkrep[0:14] = spk_c.astype(np.float16)
        spkrep[14:28] = spk_c.astype(np.float16)
        spkrep[28:42] = ((1.0 - spk_c) * mask_col).astype(np.float16)
        spkrep[64:78] = mask_col.astype(np.float16)
        coefrep = np.zeros((VROWS, COLS), dtype=np.float16)
        coefrep[0:14] = (ic0_c - ic0s_c).astype(np.float16)
        coefrep[14:28] = ic1s_c.astype(np.float16)
        coefrep[28:42] = ic0_c.astype(np.float16)

        in_maps.append(dict(
            xt=xts, wbig=Wbig, smat=S,
            spkrep=spkrep, coefrep=coefrep, constp=constp,
        ))
        unshard_info.append((nodes_real, cols_real))
    return in_maps, unshard_info


# ---------------------------------------------------------------- bass kernel
_COMPILED = None


def _build():
    import concourse.bass as bass
    from concourse import bacc
    import concourse.mybir as mybir
    from concourse.tile import TileContext

    f16 = mybir.dt.float16
    f32 = mybir.dt.float32
    ADD = mybir.AluOpType.add
    MUL = mybir.AluOpType.mult

    nc = bacc.Bacc("TRN2", target_bir_lowering=False, debug=False,
                   num_devices=NCORES)
    xt_d = nc.dram_tensor("xt", [NT, 128, KB * NTC], f16, kind="ExternalInput")
    wbig_d = nc.dram_tensor("wbig", [128, KB, M], f16, kind="ExternalInput")
    smat_d = nc.dram_tensor("smat", [128, 2, M2], f16, kind="ExternalInput")
    spkrep_d = nc.dram_tensor("spkrep", [M, COLS], f16, kind="ExternalInput")
    coefrep_d = nc.dram_tensor("coefrep", [VROWS, COLS], f16, kind="ExternalInput")
    constp_d = nc.dram_tensor("constp", [NR, GRP], f16, kind="ExternalInput")
    y_d = nc.dram_tensor("y", [NCLS, COLS], f16, kind="ExternalOutput")

    with TileContext(nc) as tc:
        with (
            tc.tile_pool(name="const", bufs=1) as cpool,
            tc.tile_pool(name="xin", bufs=8) as xpool,
            tc.tile_pool(name="wrk", bufs=3) as wpool,
            tc.tile_pool(name="g2", bufs=1) as gpool,
            tc.tile_pool(name="psum", bufs=6, space="PSUM") as ppool,
            tc.tile_pool(name="psum2", bufs=2, space="PSUM") as p2pool,
        ):
            wsb = cpool.tile([128, KB, M], f16)
            ssb = cpool.tile([128, 2, M2], f16)

            tZ = cpool.tile([M, COLS], f16)
            tT1 = cpool.tile([VROWS, COLS], f16)
            tSPK = cpool.tile([M, COLS], f16)
            tCF = cpool.tile([VROWS, COLS], f16)
            # packed plane: win2 quarter g at rows 32g:32g+7
            tGP = cpool.tile([NR, GRP], f16)
            tFP = cpool.tile([NR, GRP], f16)
            tCP = cpool.tile([NR, GRP], f16)
            tYP = cpool.tile([NR, GRP], f16)
            w1w = gpool.tile([NR, GRP], f16, name="w1w")
            w2w = gpool.tile([NR, GRP], f16, name="w2w")
            nc.scalar.dma_start(wsb[:], wbig_d[:])
            nc.scalar.dma_start(ssb[:], smat_d[:])
            nc.scalar.dma_start(tSPK[:], spkrep_d[:])
            nc.scalar.dma_start(tCF[:], coefrep_d[:])
            nc.scalar.dma_start(tCP[:], constp_d[:])

            T1 = tT1[0:VROWS]

            def win2(a, b, flush):
                """Second window over the packed g0 plane, columns [a:b),
                then y = W(g0) + f0 + const; flush DMAs cols [flush:b)."""
                gp, fp, cp, yp = tGP[:], tFP[:], tCP[:], tYP[:]
                w1, w2 = w1w[:], w2w[:]
                if a == 0:
                    nc.vector.tensor_copy(w1[:, 0:1], gp[:, 0:1])
                    nc.vector.tensor_tensor(w1[:, 1:b], gp[:, 1:b], gp[:, 0:b - 1], ADD)
                    nc.vector.tensor_copy(w2[:, 0:2], w1[:, 0:2])
                    nc.vector.tensor_tensor(w2[:, 2:b], w1[:, 2:b], w1[:, 0:b - 2], ADD)
                    nc.vector.tensor_copy(yp[:, 0:4], w2[:, 0:4])
                    nc.vector.tensor_tensor(yp[:, 4:b], w2[:, 4:b], gp[:, 0:b - 4], ADD)
                else:
                    nc.vector.tensor_tensor(w1[:, a:b], gp[:, a:b], gp[:, a - 1:b - 1], ADD)
                    nc.vector.tensor_tensor(w2[:, a:b], w1[:, a:b], w1[:, a - 2:b - 2], ADD)
                    nc.vector.tensor_tensor(yp[:, a:b], w2[:, a:b], gp[:, a - 4:b - 4], ADD)
                nc.vector.tensor_tensor(yp[:, a:b], yp[:, a:b], fp[:, a:b], ADD)
                nc.vector.tensor_tensor(yp[:, a:b], yp[:, a:b], cp[:, a:b], ADD)
                if flush is not None:
                    for g in range(4):
                        nc.sync.dma_start(
                            y_d[:, g * GRP + flush:g * GRP + b],
                            tYP[32 * g:32 * g + NCLS, flush:b])

            pend = []  # (V, c0, c1) tiles awaiting S-matmul + evac
            for t in range(NT):
                c0, c1 = t * NTC, (t + 1) * NTC
                xt_t = xpool.tile([128, KB, NTC], f16)
                if t < 2:   # halves let the first matmuls start sooner
                    nc.sync.dma_start(xt_t[:, 0:4, :], xt_d[t][:, 0:4 * NTC])
                    nc.sync.dma_start(xt_t[:, 4:8, :], xt_d[t][:, 4 * NTC:])
                else:
                    nc.sync.dma_start(xt_t[:], xt_d[t])
                # lagged-by-2 channel reduction first: fills the PE queue so
                # it keeps streaming while this tile's xt DMA completes
                if t == 1:
                    nc.scalar.dma_start(tCP[0][:], constp_d[0])
                    nc.scalar.dma_start(tCP[1][:], constp_d[1])
                if len(pend) == 2:
                    _sreduce(nc, p2pool, pend.pop(0), ssb, tZ, tGP, tFP)
                ps = ppool.tile([M, NTC], f32)
                for k in range(KB):
                    nc.tensor.matmul(ps[:], wsb[:, k, :], xt_t[:, k, :],
                                     start=(k == 0), stop=(k == KB - 1))
                if t == NT - 1:  # lag 1 for tile 11 so the tail stays short
                    _sreduce(nc, p2pool, pend.pop(0), ssb, tZ, tGP, tFP)

                # PSUM -> SBUF fp16 (Act), then per-row masks in-SBUF on
                # DVE at 2x rate (the PSUM read forces 1x, so split the work)
                nc.scalar.copy(tZ[:, c0:c1], ps[:])
                nc.vector.tensor_tensor(tZ[:, c0:c1], tZ[:, c0:c1],
                                        tSPK[:, c0:c1], MUL)

                # 5-tap causal window as a shift tree (rows 0:42):
                #   t1 = z + sh1(z); t2 = t1 + sh2(t1); wt = t2 + sh4(z)
                T2 = wpool.tile([WROWS, NTC], f16, tag="T2")
                WT = wpool.tile([WROWS, NTC], f16, tag="WT")
                V = wpool.tile([WROWS, NTC], f16, tag="V")
                Zw = tZ[0:WROWS]
                if t == 0:
                    nc.vector.tensor_copy(T1[:, 0:1], Zw[:, 0:1])
                    nc.vector.tensor_tensor(T1[:, 1:c1], Zw[:, 1:c1], Zw[:, 0:c1 - 1], ADD)
                    nc.vector.tensor_copy(T2[:, 0:2], T1[:, 0:2])
                    nc.vector.tensor_tensor(T2[:, 2:], T1[:, 2:c1], T1[:, 0:c1 - 2], ADD)
                    nc.vector.tensor_copy(WT[:, 0:4], T2[:, 0:4])
                    nc.vector.tensor_tensor(WT[:, 4:], T2[:, 4:], Zw[:, 0:c1 - 4], ADD)
                else:
                    nc.vector.tensor_tensor(T1[:, c0:c1], Zw[:, c0:c1], Zw[:, c0 - 1:c1 - 1], ADD)
                    nc.vector.tensor_tensor(T2[:], T1[:, c0:c1], T1[:, c0 - 2:c1 - 2], ADD)
                    nc.vector.tensor_tensor(WT[:], T2[:], Zw[:, c0 - 4:c1 - 4], ADD)
                nc.vector.tensor_tensor(V[:], WT[:], tCF[:, c0:c1], MUL)
                pend.append((V, c0, c1))

                # rolling second-window chunks over whichever packed columns
                # the lagged evacuations have completed
                if t == 5:
                    win2(0, 0, 384, 0)
                elif t == 6:
                    win2(0, 384, 896, 384)
                elif t == 7:
                    win2(0, 896, 1408, 896)
                elif t == 8:
                    win2(0, 1408, GRP, 1408)
                elif t == 11:
                    win2(1, 0, 128, None)
                elif t == 12:
                    win2(1, 128, 1152, 0)
            _sreduce(nc, p2pool, pend.pop(0), ssb, tZ, tGP, tFP)
            win2(1, 1152, GRP, 1152)
    nc.compile()
    return nc


def _sreduce(nc, p2pool, prev, ssb, tZ, tGP, tFP):
    """Lagged channel reduction for the previous tile: V + Z-roots -> ps2,
    then Act evacuates ps2 into the packed g0/f0 plane."""
    import concourse.mybir as mybir
    f32 = mybir.dt.float32
    V, c0, c1 = prev
    ps2 = p2pool.tile([M2, NTC], f32, name="ps2")
    nc.tensor.matmul(ps2[:], ssb[0:VROWS, 0, :], V[:], start=True, stop=False)
    nc.tensor.matmul(ps2[:], ssb[0:M, 1, :], tZ[:, c0:c1], start=False, stop=True)
    lo_g, hi_g = c0 // GRP, (c1 - 1) // GRP
    for g in range(lo_g, hi_g + 1):
        glo, ghi = max(c0, g * GRP), min(c1, (g + 1) * GRP)
        dst = slice(glo - g * GRP, ghi - g * GRP)
        src = slice(glo - c0, ghi - c0)
        nc.scalar.copy(tGP[32 * g:32 * g + NCLS, dst], ps2[0:NCLS, src])
        nc.scalar.copy(tFP[32 * g:32 * g + NCLS, dst], ps2[32:32 + NCLS, src])


def _get_compiled():
    global _COMPILED
    if _COMPILED is None:
        _COMPILED = _build()
    return _COMPILED


def _run(in_maps, trace=False):
    from concourse.bass_utils import run_bass_kernel_spmd
    nc = _get_compiled()
    return run_bass_kernel_spmd(nc, in_maps, list(range(NCORES)), trace=trace)


def kernel(**inputs) -> np.ndarray:
    in_maps, unshard_info = _host_prep(**inputs)
    res = _run(in_maps)
    out = np.zeros((N, NCLS), dtype=np.float32)
    for c in range(NCORES):
        nodes_real, cols_real = unshard_info[c]
        out[nodes_real] = res.results[c]["y"][:, cols_real].T.astype(np.float32)
    return out
